# revision 2
# baseline (speedup 1.0000x reference)
"""Trainium2 Bass kernel for nn_LinearLayer_45243185496808.

Computes out[b,o] = sum_i tanh(x[b,i]*t) * (sum_p coef[o,i,p]) with
B=131072, I=O=128, P_NUM=16, data-parallel over batch on 8 NeuronCores.

v2 design (vs the 40us v1): the kernel is elementwise-walled — ScalarE
tanh (1x @1.2GHz) plus the PSUM->SBUF fp32 eviction (1x on DVE @0.96GHz)
are ~16us of combined engine time that nothing else can absorb (GpSimd
has no PSUM port, DMA can't read PSUM, TRN2 matmul output must be fp32).
Everything else is arranged so those two engines start as early as
possible and never wait:

  - x ships as fp8e4 [i=128, b] (pure transpose, 1B/elem): halves load
    bytes to 2 MiB/core so DMA is never in contention, and ACTIVATE
    reads fp8 directly at the same 1x rate. Measured end-to-end rel err
    ~1.5e-2 vs the 2e-2 gate (CPU sim matches device numerics closely).
  - transposed output layout: wT [i,o] f16 is the PE stationary, tanh
    values [i, b] stream as the moving operand in N=512 column blocks
    (one PSUM bank per matmul, 4 banks per PSUM tile). One weight load
    conceptually; no per-slice LDWEIGHTS churn; PE time ~8us, off the
    critical path. PSUM holds out.T [o, bcols]; evictions write out.T
    tiles that store as contiguous [128, W] runs; host transposes.
  - identity (for the w = sum_p coef reduction via 16 accumulating
    identity matmuls) rides in the coef DMA instead of being built by
    gpsimd memsets, so gpsimd's first action is issuing the chunk-0
    load (~t0+0.1us) and ScalarE's first action is the tanh of chunk 0
    (the ACT table load runs eagerly at t0, concurrent with the load).
  - eviction split: DVE takes most PSUM->SBUF casts; ScalarE takes the
    tail chunks' (its tanh stream ends first) so both engines drain
    together.
  - measured exec window = first kernel instruction -> end of the
    framework's fixed per-engine semaphore-reset storm; nothing to do
    about the storm itself, but a shorter kernel keeps HAM clocks high
    through it.

HBM per core: 2 MiB x(fp8) + 0.53 MiB coef+identity + 4 MiB out(f16).
"""

import os
import sys
import types

import ml_dtypes
import numpy as np

import concourse.bass as bass
import concourse.mybir as mybir
import concourse.tile as tile
from concourse import bacc
from concourse.bass_utils import run_bass_kernel_spmd


def _ensure_ntff_hook():
    """Register the axon NTFF profile hook if the image lacks antenv.axon_hooks.

    Only needed for BASS_TRACE=1 profiling runs; harmless otherwise."""
    if "antenv.axon_hooks" in sys.modules:
        return
    try:
        from antenv.axon_hooks import get_axon_ntff_profile_hook  # noqa: F401

        return  # real module importable
    except ImportError:
        pass
    hook = None
    try:
        from trn_agent_boot.trn_boot import _ntff_profile_via_ctypes

        so_path = "/opt/axon/libaxon_pjrt.so"
        if os.path.exists(so_path):
            hook = _ntff_profile_via_ctypes(so_path)
    except Exception:
        hook = None
    mod = types.ModuleType("antenv.axon_hooks")
    mod.get_axon_ntff_profile_hook = lambda: hook
    mod.set_axon_ntff_profile_hook = lambda h: None
    sys.modules["antenv.axon_hooks"] = mod


N_CORES = 8
B_FULL = 131072
I_DIM = 128
O_DIM = 128
P_NUM = 16
P = 128                     # SBUF partitions
B_CORE = B_FULL // N_CORES  # 16384

# Load chunks of xt == tanh tiles (base, width). Small leading chunk gets
# the tanh stream started ASAP; small trailing chunk shortens the drain.
CHUNKS = [(0, 1024), (1024, 2048), (3072, 4096), (7168, 4096),
          (11264, 4096), (15360, 1024)]
assert CHUNKS[-1][0] + CHUNKS[-1][1] == B_CORE
assert all(a + w == b for (a, w), (b, _) in zip(CHUNKS, CHUNKS[1:]))

NMM = 512                   # moving cols per matmul = one PSUM bank of f32
QUAD = 4 * NMM              # cols per PSUM tile (4 banks)

# coefT layout: [identity(128) | p-major coef blocks (16 x 128)]
CW = O_DIM * P_NUM          # 2048
COEF_COLS = 128 + CW        # 2176

LAST_RESULT = None  # BassKernelResults of the most recent run (for test.py)


def build_bass(tanh_scale: float) -> bass.Bass:
    nc = bacc.Bacc("TRN2", target_bir_lowering=False)
    xt = nc.dram_tensor("xt", [P, B_CORE], mybir.dt.float8e4, kind="ExternalInput")
    coefT = nc.dram_tensor(
        "coefT", [I_DIM, COEF_COLS], mybir.dt.float16, kind="ExternalInput"
    )
    outT = nc.dram_tensor("outT", [P, B_CORE], mybir.dt.float16, kind="ExternalOutput")

    with tile.TileContext(nc) as tc:
        with (
            tc.tile_pool(name="consts", bufs=1) as consts,
            tc.tile_pool(name="xin", bufs=1) as xin_pool,
            tc.tile_pool(name="vals", bufs=3) as vals_pool,
            tc.tile_pool(name="outp", bufs=3) as out_pool,
            tc.tile_pool(name="pout", bufs=2, space="PSUM") as pout_pool,
        ):
            # Scratch weights for PE clock (HAM) warmup: one cheap memset,
            # then gpsimd immediately turns to issuing the chunk-0 load.
            warm = consts.tile([P, P], mybir.dt.float16)
            nc.gpsimd.memset(warm[:], 0.0)

            x_tiles = [None] * len(CHUNKS)

            def load_chunk(ci, eng):
                base, wcols = CHUNKS[ci]
                x_sb = xin_pool.tile([P, wcols], mybir.dt.float8e4, tag=f"x{ci}")
                eng.dma_start(out=x_sb[:], in_=xt[:, base : base + wcols])
                x_tiles[ci] = x_sb

            # chunk 0 rides gpsimd (ready earliest); coef halves next
            # (identity travels with the first half), then the remaining
            # chunks on the sync HWDGE ring.
            load_chunk(0, nc.gpsimd)
            coef_sb = consts.tile([P, COEF_COLS], mybir.dt.float16)
            half = COEF_COLS // 2  # 1088
            nc.gpsimd.dma_start(out=coef_sb[:, :half], in_=coefT[:, :half])
            nc.sync.dma_start(out=coef_sb[:, half:], in_=coefT[:, half:])
            for ci in range(1, len(CHUNKS)):
                load_chunk(ci, nc.sync)

            identity_h = coef_sb[:, :P]

            # PE warmup on the zero scratch while the DMAs fly.
            wm_ps = pout_pool.tile([P, QUAD], mybir.dt.float32, tag="o_ps")
            for wi in range(16):
                nc.tensor.matmul(
                    wm_ps[:, (wi % 4) * NMM : (wi % 4) * NMM + P],
                    warm[:],
                    warm[:],
                    start=True,
                    stop=True,
                )

            # wT[i,o] = sum_p coef via 16 identity matmuls accumulating in
            # PSUM (I.T @ block_p = block_p), then one DVE cast to f16.
            w_big = pout_pool.tile([P, QUAD], mybir.dt.float32, tag="o_ps")
            w_ps = w_big[:, :O_DIM]
            for k in range(P_NUM):
                nc.tensor.matmul(
                    w_ps,
                    identity_h,
                    coef_sb[:, P + k * O_DIM : P + (k + 1) * O_DIM],
                    start=(k == 0),
                    stop=(k == P_NUM - 1),
                )
            wT = consts.tile([P, O_DIM], mybir.dt.float16)
            nc.vector.tensor_copy(wT[:], w_ps)

            # --- main loop ---
            # Per chunk: tanh -> per-QUAD psum tiles of out.T -> evict ->
            # store. DVE evicts everything except the last two chunks'
            # quads, which ScalarE takes once its tanh stream is done.
            for ci, (base, wcols) in enumerate(CHUNKS):
                v_sb = vals_pool.tile([P, wcols], mybir.dt.float16, tag="v_sb")
                nc.scalar.activation(
                    v_sb[:],
                    x_tiles[ci][:],
                    mybir.ActivationFunctionType.Tanh,
                    scale=tanh_scale,
                )
                out_sb = out_pool.tile([P, wcols], mybir.dt.float16, tag="out_sb")
                for g0 in range(0, wcols, QUAD):
                    gw = min(QUAD, wcols - g0)
                    o_ps = pout_pool.tile([P, gw], mybir.dt.float32, tag="o_ps")
                    for j0 in range(0, gw, NMM):
                        jw = min(NMM, gw - j0)
                        nc.tensor.matmul(
                            o_ps[:, j0 : j0 + jw],
                            wT[:],
                            v_sb[:, g0 + j0 : g0 + j0 + jw],
                            start=True,
                            stop=True,
                        )
                    if ci >= len(CHUNKS) - 2:
                        nc.scalar.copy(out_sb[:, g0 : g0 + gw], o_ps[:])
                    else:
                        nc.vector.tensor_copy(out_sb[:, g0 : g0 + gw], o_ps[:])
                out_view = outT[:, base : base + wcols]
                (nc.sync if ci >= len(CHUNKS) - 2 else nc.gpsimd).dma_start(
                    out=out_view, in_=out_sb[:]
                )
    nc.finalize()
    return nc


def kernel(x, coef, tanh_range):
    global LAST_RESULT
    x = np.asarray(x, dtype=np.float32)
    coef = np.asarray(coef, dtype=np.float32)
    t = float(np.asarray(tanh_range))
    assert x.shape == (B_FULL, I_DIM), x.shape
    assert coef.shape == (O_DIM, I_DIM, P_NUM), coef.shape

    # [identity | p-major coef blocks]: block p is the [i, o] slice.
    coefT = np.empty((I_DIM, COEF_COLS), dtype=np.float16)
    coefT[:, :P] = np.eye(P, dtype=np.float16)
    coefT[:, P:] = (
        coef.transpose(1, 2, 0).astype(np.float16).reshape(I_DIM, CW)
    )
    nc = build_bass(t)
    xt_full = np.ascontiguousarray(x.T).astype(ml_dtypes.float8_e4m3)
    in_maps = [
        {"xt": np.ascontiguousarray(xt_full[:, k * B_CORE : (k + 1) * B_CORE]),
         "coefT": coefT}
        for k in range(N_CORES)
    ]
    if os.environ.get("BASS_TRACE"):
        _ensure_ntff_hook()
    res = run_bass_kernel_spmd(nc, in_maps, core_ids=list(range(N_CORES)))
    LAST_RESULT = res
    return np.concatenate(
        [r["outT"].astype(np.float32).T for r in res.results], axis=0
    )
